# revision 12
# baseline (speedup 1.0000x reference)
"""Trainium2 Bass kernel for 3-layer per-task LoRA MLP.

Full-input contract: kernel(**inputs) takes the unsharded tensors and returns
the full [8, 1024, 1024] output. Internally the task axis (t=8) is sharded
across 8 NeuronCores (one task per core).

Strategy:
  - LoRA is folded on the host into per-task effective weights
    W_eff = k + (alpha/r) * d @ u  (standard LoRA weight merging), so the
    device kernel is a plain 3-layer MLP — no rank-8 matmuls on the PE.
  - weights and activations are bf16 on device (1 cycle/row on the PE, same
    as f32r, but half the DMA traffic and SBUF footprint); PSUM accumulation
    stays f32. Measured pipeline error ~4e-3 relative.
  - x is pre-transposed on the host so activations live as h^T
    [feat(part), batch(free)] with zero on-device transposes; the final
    layer uses h2^T as the *stationary* operand and w2 as the moving
    operand, producing natural-layout [batch, feat] output directly.
  - weights are pre-packed on the host into the exact SBUF tile layout so
    every DMA is >=2KB-contiguous per partition.
  - layer-2 bias arrives pre-broadcast [128, H3] and is added by the DVE
    while draining PSUM; layer-0/1 biases ride the activation instruction.
"""

import sys

if "/opt/trn_rl_repo" not in sys.path:
    sys.path.insert(0, "/opt/trn_rl_repo")

import numpy as np

T, B, D = 8, 1024, 1024
H1, H2, H3 = 2048, 2048, 1024
SCALING = 2.0  # alpha/rank = 16/8
P = 128
NT = 512  # PSUM free-dim tile (fp32 one-bank limit)

_CACHE = {}


def _build():
    import concourse.mybir as mybir
    from concourse import bacc
    from concourse.tile import TileContext
    from concourse.bass import ts

    f32 = mybir.dt.float32
    bf16 = mybir.dt.bfloat16
    AF = mybir.ActivationFunctionType

    nc = bacc.Bacc(None, target_bir_lowering=False, name="lora_mlp")

    KT0 = D // P      # 8  k-tiles, layer 0
    KT1 = H1 // P     # 16 k-tiles, layer 1
    KT2 = H2 // P     # 16 k-tiles, layer 2
    MT0 = H1 // P     # 16 m-tiles, layer 0
    MT1 = H2 // P     # 16 m-tiles, layer 1
    BT = B // P       # 8  batch 128-tiles
    NB = B // NT      # 2  batch 512-halves (free dim, layers 0/1)
    N2 = H3 // NT     # 2  feature 512-halves (free dim, layer 2)

    xt = nc.dram_tensor("xt", (D, B), bf16, kind="ExternalInput")
    w0 = nc.dram_tensor("w0", (MT0, P, KT0 * P), bf16, kind="ExternalInput")
    b0 = nc.dram_tensor("b0", (P, MT0), f32, kind="ExternalInput")
    w1 = nc.dram_tensor("w1", (MT1, P, KT1 * P), bf16, kind="ExternalInput")
    b1 = nc.dram_tensor("b1", (P, MT1), f32, kind="ExternalInput")
    w2 = nc.dram_tensor("w2", (H2, H3), bf16, kind="ExternalInput")
    b2 = nc.dram_tensor("b2", (P, H3), f32, kind="ExternalInput")
    out = nc.dram_tensor("out", (B, H3), f32, kind="ExternalOutput")

    with TileContext(nc) as tc:
        with (
            tc.tile_pool(name="main", bufs=1) as pool,
            tc.tile_pool(name="psum", bufs=1, space="PSUM") as pp,
        ):
            # PE warmup: the tensor engine clock ramps with sustained use
            # (0.65 -> 1.2 -> 2.4 GHz over ~3us). Run throwaway matmuls on a
            # memset tile while the first input DMAs land, so the real
            # matmuls start at full clock.
            wu = pool.tile([P, 2 * P], bf16, tag="wu", bufs=1)
            nc.vector.memset(wu, 0.125)
            wps = pp.tile([P, 2 * P], f32, tag="wps", bufs=1)
            for _ in range(12):
                nc.tensor.matmul(wps, wu[:, :P], wu, start=True, stop=True)

            # x^T lives as 16 half-tiles [128, 512], one DMA each (consumers
            # gate on whole-tile writes, so one-DMA-per-tile keeps the first
            # PSUM groups from waiting on later chunks). Loads are spread
            # over the three DMA channels (SP HWDGE, Act HWDGE, Pool SWDGE);
            # the Act queue starts ~1.5us late behind LoadActFuncSet.
            xh = [
                [
                    pool.tile([P, NT], bf16, tag="X", bufs=2 * KT0, name=f"xh{k}_{n}")
                    for n in range(NB)
                ]
                for k in range(KT0)
            ]
            w0t = [None]  # m=0 uses the split w0ta/w0tb tiles below
            for m in range(1, MT0):
                w0t.append(pool.tile([P, KT0 * P], bf16, tag="W0", bufs=6, name=f"w0t{m}"))
            # first m-tile of w0 split in two so k=0..3 don't wait on k=4..7
            w0ta = pool.tile([P, 4 * P], bf16, tag="W0a", bufs=1)
            w0tb = pool.tile([P, 4 * P], bf16, tag="W0b", bufs=1)

            def xdma(eng, k, n):
                eng.dma_start(out=xh[k][n], in_=xt[ts(k, P), ts(n, NT)])

            with tc.high_priority():
                # first halves (n=0): gate the very first PSUM group.
                # Only SP + Pool here — the Act HWDGE queue starts ~1.5us
                # late (LoadActFuncSet) and the scheduler hoists other
                # loads onto it.
                nc.sync.dma_start(out=w0ta, in_=w0[0, :, 0 : 4 * P])
                nc.gpsimd.dma_start(out=w0tb, in_=w0[0, :, 4 * P : 8 * P])
                xdma(nc.sync, 0, 0)
                xdma(nc.gpsimd, 3, 0)
                xdma(nc.sync, 1, 0)
                xdma(nc.gpsimd, 4, 0)
                xdma(nc.sync, 2, 0)
                xdma(nc.gpsimd, 5, 0)
                xdma(nc.gpsimd, 6, 0)
                xdma(nc.gpsimd, 7, 0)
                # second halves (n=1)
                xdma(nc.sync, 0, 1)
                xdma(nc.sync, 1, 1)
                xdma(nc.sync, 2, 1)
                xdma(nc.sync, 3, 1)
                xdma(nc.gpsimd, 4, 1)
                xdma(nc.gpsimd, 5, 1)
                xdma(nc.gpsimd, 6, 1)
                xdma(nc.gpsimd, 7, 1)
                nc.sync.dma_start(out=w0t[1], in_=w0[1])
            b0_sb = pool.tile([P, MT0], f32, tag="b0", bufs=1)
            nc.gpsimd.dma_start(out=b0_sb, in_=b0[:, :])
            b1_sb = pool.tile([P, MT1], f32, tag="b1", bufs=1)
            nc.gpsimd.dma_start(out=b1_sb, in_=b1[:, :])
            b2_sb = pool.tile([P, H3], f32, tag="b2", bufs=1)

            # =================== layer 0 ===================
            # k-accumulation order of the first two groups follows DMA
            # arrival order; later groups have everything resident.
            korder0 = [0, 3, 1, 4, 2, 5, 6, 7]
            korder1 = [0, 1, 4, 2, 5, 3, 6, 7]
            h0T = []
            for m in range(MT0):
                wt = w0t[m]
                if m >= 2:
                    nc.sync.dma_start(out=wt, in_=w0[m])
                ht = pool.tile([P, B], bf16, tag="H0", bufs=MT0)
                h0T.append(ht)
                for n in range(NB):
                    ps = pp.tile([P, NT], f32, tag="pm", bufs=6)
                    if m == 0:
                        ks = korder0 if n == 0 else korder1
                    else:
                        ks = range(KT0)
                    for i, k in enumerate(ks):
                        if m == 0:
                            stat = (w0ta if k < 4 else w0tb)[:, ts(k % 4, P)]
                        else:
                            stat = wt[:, ts(k, P)]
                        nc.tensor.matmul(
                            ps,
                            stat,
                            xh[k][n],
                            start=(i == 0),
                            stop=(i == KT0 - 1),
                        )
                    nc.scalar.activation(
                        ht[:, ts(n, NT)], ps, AF.Relu, bias=b0_sb[:, ts(m, 1)]
                    )

            # =================== layer 1 ===================
            h1T = []
            for m in range(MT1):
                wt = pool.tile([P, KT1 * P], bf16, tag="W1", bufs=6)
                nc.sync.dma_start(out=wt, in_=w1[m])
                ht = pool.tile([P, B], bf16, tag="H1", bufs=MT1)
                h1T.append(ht)
                for n in range(NB):
                    ps = pp.tile([P, NT], f32, tag="pm", bufs=6)
                    for k in range(KT1):
                        nc.tensor.matmul(
                            ps,
                            wt[:, ts(k, P)],
                            h0T[k][:, ts(n, NT)],
                            start=(k == 0),
                            stop=(k == KT1 - 1),
                        )
                    nc.scalar.activation(
                        ht[:, ts(n, NT)], ps, AF.Relu, bias=b1_sb[:, ts(m, 1)]
                    )

            # =================== layer 2 (natural output) ===================
            # w2 streams on the Activation-engine HWDGE queue so it never
            # queues behind the slot-paced w0/w1 stream on the sync queue.
            k2t = []
            for k in range(KT2):
                kt_ = pool.tile([P, H3], bf16, tag="K2", bufs=KT2, name=f"k2t{k}")
                k2t.append(kt_)
                nc.scalar.dma_start(out=kt_, in_=w2[ts(k, P), :])
            nc.scalar.dma_start(out=b2_sb, in_=b2[:, :])
            for m in range(BT):
                ot = pool.tile([P, H3], f32, tag="O", bufs=4)
                # last m-tile runs in 256-wide chunks so the final
                # DVE-add + store chain after the last matmul is short
                nchunks, cw = (N2, NT) if m < BT - 1 else (4, NT // 2)
                for n in range(nchunks):
                    ps = pp.tile([P, cw], f32, tag="pm", bufs=6)
                    for k in range(KT2):
                        nc.tensor.matmul(
                            ps,
                            h1T[k][:, ts(m, P)],
                            k2t[k][:, ts(n, cw)],
                            start=(k == 0),
                            stop=(k == KT2 - 1),
                        )
                    nc.vector.tensor_add(ot[:, ts(n, cw)], ps, b2_sb[:, ts(n, cw)])
                    nc.scalar.dma_start(
                        out=out[ts(m, P), ts(n, cw)], in_=ot[:, ts(n, cw)]
                    )

    if not nc.is_finalized():
        nc.finalize()
    return nc


def _get_nc():
    if "nc" not in _CACHE:
        _CACHE["nc"] = _build()
    return _CACHE["nc"]


def _task_in_map(inputs, t, bf16, b0c, b1c, b2c):
    W0 = inputs["k0"] + SCALING * (inputs["d0"][:, :, t] @ inputs["u0"][:, :, t])
    W1 = inputs["k1"] + SCALING * (inputs["d1"][:, :, t] @ inputs["u1"][:, :, t])
    W2 = inputs["k2"] + SCALING * (inputs["d2"][:, :, t] @ inputs["u2"][:, :, t])
    # pack [K, M] -> [m, p, k*128+c] with element (m,p,kc) = W[k*128+p, m*128+c]
    w0r = np.ascontiguousarray(
        W0.reshape(8, 128, 16, 128).transpose(2, 1, 0, 3).reshape(16, 128, 1024),
        dtype=bf16,
    )
    w1r = np.ascontiguousarray(
        W1.reshape(16, 128, 16, 128).transpose(2, 1, 0, 3).reshape(16, 128, 2048),
        dtype=bf16,
    )
    w2r = np.ascontiguousarray(W2, dtype=bf16)
    xtr = np.ascontiguousarray(inputs["x"][t].T, dtype=bf16)
    return {
        "xt": xtr,
        "w0": w0r,
        "b0": b0c,
        "w1": w1r,
        "b1": b1c,
        "w2": w2r,
        "b2": b2c,
    }


def build_in_maps(inputs):
    import concurrent.futures

    import ml_dtypes

    bf16 = ml_dtypes.bfloat16
    b0c = np.ascontiguousarray(inputs["b0"].reshape(16, 128).T, dtype=np.float32)
    b1c = np.ascontiguousarray(inputs["b1"].reshape(16, 128).T, dtype=np.float32)
    b2c = np.ascontiguousarray(
        np.broadcast_to(inputs["b2"], (P, H3)), dtype=np.float32
    )
    with concurrent.futures.ThreadPoolExecutor(max_workers=T) as ex:
        in_maps = list(
            ex.map(lambda t: _task_in_map(inputs, t, bf16, b0c, b1c, b2c), range(T))
        )
    return in_maps


def kernel(**inputs):
    from concourse import bass_utils

    nc = _get_nc()
    in_maps = build_in_maps(inputs)
    res = bass_utils.run_bass_kernel_spmd(nc, in_maps, core_ids=list(range(T)))
    return np.stack([r["out"] for r in res.results], axis=0)


# revision 13
# speedup vs baseline: 19523.8741x; 19523.8741x over previous
"""Trainium2 Bass kernel for 3-layer per-task LoRA MLP.

Full-input contract: kernel(**inputs) takes the unsharded tensors and returns
the full [8, 1024, 1024] output. Internally the task axis (t=8) is sharded
across 8 NeuronCores (one task per core).

Strategy:
  - LoRA is folded on the host into per-task effective weights
    W_eff = k + (alpha/r) * d @ u  (standard LoRA weight merging), so the
    device kernel is a plain 3-layer MLP — no rank-8 matmuls on the PE.
  - weights and activations are bf16 on device (1 cycle/row on the PE, same
    as f32r, but half the DMA traffic and SBUF footprint); PSUM accumulation
    stays f32. Measured pipeline error ~4e-3 relative.
  - x is pre-transposed on the host so activations live as h^T
    [feat(part), batch(free)] with zero on-device transposes; the final
    layer uses h2^T as the *stationary* operand and w2 as the moving
    operand, producing natural-layout [batch, feat] output directly.
  - weights are pre-packed on the host into the exact SBUF tile layout so
    every DMA is >=2KB-contiguous per partition.
  - layer-2 bias arrives pre-broadcast [128, H3] and is added by the DVE
    while draining PSUM; layer-0/1 biases ride the activation instruction.
"""

import sys

if "/opt/trn_rl_repo" not in sys.path:
    sys.path.insert(0, "/opt/trn_rl_repo")

import numpy as np

T, B, D = 8, 1024, 1024
H1, H2, H3 = 2048, 2048, 1024
SCALING = 2.0  # alpha/rank = 16/8
P = 128
NT = 512  # PSUM free-dim tile (fp32 one-bank limit)

_CACHE = {}


def _build():
    import concourse.mybir as mybir
    from concourse import bacc
    from concourse.tile import TileContext
    from concourse.bass import ts

    f32 = mybir.dt.float32
    bf16 = mybir.dt.bfloat16
    AF = mybir.ActivationFunctionType

    nc = bacc.Bacc(None, target_bir_lowering=False, name="lora_mlp")

    KT0 = D // P      # 8  k-tiles, layer 0
    KT1 = H1 // P     # 16 k-tiles, layer 1
    KT2 = H2 // P     # 16 k-tiles, layer 2
    MT0 = H1 // P     # 16 m-tiles, layer 0
    MT1 = H2 // P     # 16 m-tiles, layer 1
    BT = B // P       # 8  batch 128-tiles
    NB = B // NT      # 2  batch 512-halves (free dim, layers 0/1)
    N2 = H3 // NT     # 2  feature 512-halves (free dim, layer 2)

    xt = nc.dram_tensor("xt", (D, B), bf16, kind="ExternalInput")
    w0 = nc.dram_tensor("w0", (MT0, P, KT0 * P), bf16, kind="ExternalInput")
    b0 = nc.dram_tensor("b0", (P, MT0), f32, kind="ExternalInput")
    w1 = nc.dram_tensor("w1", (MT1, P, KT1 * P), bf16, kind="ExternalInput")
    b1 = nc.dram_tensor("b1", (P, MT1), f32, kind="ExternalInput")
    w2 = nc.dram_tensor("w2", (H2, H3), bf16, kind="ExternalInput")
    b2 = nc.dram_tensor("b2", (P, H3), f32, kind="ExternalInput")
    out = nc.dram_tensor("out", (B, H3), f32, kind="ExternalOutput")

    with TileContext(nc) as tc:
        with (
            tc.tile_pool(name="main", bufs=1) as pool,
            tc.tile_pool(name="psum", bufs=1, space="PSUM") as pp,
        ):
            # PE warmup: the tensor engine clock ramps with sustained use
            # (0.65 -> 1.2 -> 2.4 GHz over ~3us). Run throwaway matmuls on a
            # memset tile while the first input DMAs land, so the real
            # matmuls start at full clock.
            wu = pool.tile([P, 2 * P], bf16, tag="wu", bufs=1)
            nc.vector.memset(wu, 0.125)
            wps = pp.tile([P, 2 * P], f32, tag="wps", bufs=1)
            for _ in range(10):
                nc.tensor.matmul(wps, wu[:, :P], wu, start=True, stop=True)

            # x^T lives as 16 half-tiles [128, 512], one DMA each (consumers
            # gate on whole-tile writes, so one-DMA-per-tile keeps the first
            # PSUM groups from waiting on later chunks). Loads are spread
            # over the three DMA channels (SP HWDGE, Act HWDGE, Pool SWDGE);
            # the Act queue starts ~1.5us late behind LoadActFuncSet.
            xh = [
                [
                    pool.tile([P, NT], bf16, tag="X", bufs=2 * KT0, name=f"xh{k}_{n}")
                    for n in range(NB)
                ]
                for k in range(KT0)
            ]
            w0t = [None]  # m=0 uses the split w0ta/w0tb tiles below
            for m in range(1, MT0):
                w0t.append(pool.tile([P, KT0 * P], bf16, tag="W0", bufs=6, name=f"w0t{m}"))
            # first m-tile of w0 split in two so k=0..3 don't wait on k=4..7
            w0ta = pool.tile([P, 4 * P], bf16, tag="W0a", bufs=1)
            w0tb = pool.tile([P, 4 * P], bf16, tag="W0b", bufs=1)

            def xdma(eng, k, n):
                eng.dma_start(out=xh[k][n], in_=xt[ts(k, P), ts(n, NT)])

            with tc.high_priority():
                # first halves (n=0): gate the very first PSUM group.
                # Only SP + Pool here — the Act HWDGE queue starts ~1.5us
                # late (LoadActFuncSet) and the scheduler hoists other
                # loads onto it.
                nc.sync.dma_start(out=w0ta, in_=w0[0, :, 0 : 4 * P])
                nc.gpsimd.dma_start(out=w0tb, in_=w0[0, :, 4 * P : 8 * P])
                xdma(nc.sync, 0, 0)
                xdma(nc.gpsimd, 3, 0)
                xdma(nc.sync, 1, 0)
                xdma(nc.gpsimd, 4, 0)
                xdma(nc.sync, 2, 0)
                xdma(nc.gpsimd, 5, 0)
                xdma(nc.gpsimd, 6, 0)
                xdma(nc.gpsimd, 7, 0)
                # second halves (n=1)
                xdma(nc.sync, 0, 1)
                xdma(nc.sync, 1, 1)
                xdma(nc.sync, 2, 1)
                xdma(nc.sync, 3, 1)
                xdma(nc.gpsimd, 4, 1)
                xdma(nc.gpsimd, 5, 1)
                xdma(nc.gpsimd, 6, 1)
                xdma(nc.gpsimd, 7, 1)
                nc.sync.dma_start(out=w0t[1], in_=w0[1])
            b0_sb = pool.tile([P, MT0], f32, tag="b0", bufs=1)
            nc.gpsimd.dma_start(out=b0_sb, in_=b0[:, :])
            b1_sb = pool.tile([P, MT1], f32, tag="b1", bufs=1)
            nc.gpsimd.dma_start(out=b1_sb, in_=b1[:, :])
            b2_sb = pool.tile([P, H3], f32, tag="b2", bufs=1)

            # =================== layer 0 ===================
            # k-accumulation order of the first two groups follows DMA
            # arrival order; later groups have everything resident.
            korder0 = [0, 3, 1, 4, 2, 5, 6, 7]
            korder1 = [0, 1, 4, 2, 5, 3, 6, 7]
            h0T = []
            for m in range(MT0):
                wt = w0t[m]
                if m >= 2:
                    nc.sync.dma_start(out=wt, in_=w0[m])
                ht = pool.tile([P, B], bf16, tag="H0", bufs=MT0)
                h0T.append(ht)
                for n in range(NB):
                    ps = pp.tile([P, NT], f32, tag="pm", bufs=6)
                    if m == 0:
                        ks = korder0 if n == 0 else korder1
                    else:
                        ks = range(KT0)
                    for i, k in enumerate(ks):
                        if m == 0:
                            stat = (w0ta if k < 4 else w0tb)[:, ts(k % 4, P)]
                        else:
                            stat = wt[:, ts(k, P)]
                        nc.tensor.matmul(
                            ps,
                            stat,
                            xh[k][n],
                            start=(i == 0),
                            stop=(i == KT0 - 1),
                        )
                    nc.scalar.activation(
                        ht[:, ts(n, NT)], ps, AF.Relu, bias=b0_sb[:, ts(m, 1)]
                    )

            # =================== layer 1 ===================
            h1T = []
            for m in range(MT1):
                wt = pool.tile([P, KT1 * P], bf16, tag="W1", bufs=6)
                nc.sync.dma_start(out=wt, in_=w1[m])
                ht = pool.tile([P, B], bf16, tag="H1", bufs=MT1)
                h1T.append(ht)
                for n in range(NB):
                    ps = pp.tile([P, NT], f32, tag="pm", bufs=6)
                    for k in range(KT1):
                        nc.tensor.matmul(
                            ps,
                            wt[:, ts(k, P)],
                            h0T[k][:, ts(n, NT)],
                            start=(k == 0),
                            stop=(k == KT1 - 1),
                        )
                    nc.scalar.activation(
                        ht[:, ts(n, NT)], ps, AF.Relu, bias=b1_sb[:, ts(m, 1)]
                    )

            # =================== layer 2 (natural output) ===================
            # w2 streams on the Activation-engine HWDGE queue so it never
            # queues behind the slot-paced w0/w1 stream on the sync queue.
            k2t = []
            for k in range(KT2):
                kt_ = pool.tile([P, H3], bf16, tag="K2", bufs=KT2, name=f"k2t{k}")
                k2t.append(kt_)
                nc.scalar.dma_start(out=kt_, in_=w2[ts(k, P), :])
            nc.scalar.dma_start(out=b2_sb, in_=b2[:, :])
            for m in range(BT):
                ot = pool.tile([P, H3], f32, tag="O", bufs=4)
                # last m-tile runs in 256-wide chunks so the final
                # DVE-add + store chain after the last matmul is short
                nchunks, cw = (N2, NT) if m < BT - 1 else (8, H3 // 8)
                for n in range(nchunks):
                    ps = pp.tile([P, cw], f32, tag="pm", bufs=6)
                    for k in range(KT2):
                        nc.tensor.matmul(
                            ps,
                            h1T[k][:, ts(m, P)],
                            k2t[k][:, ts(n, cw)],
                            start=(k == 0),
                            stop=(k == KT2 - 1),
                        )
                    nc.vector.tensor_add(ot[:, ts(n, cw)], ps, b2_sb[:, ts(n, cw)])
                    nc.scalar.dma_start(
                        out=out[ts(m, P), ts(n, cw)], in_=ot[:, ts(n, cw)]
                    )

    if not nc.is_finalized():
        nc.finalize()
    return nc


def _get_nc():
    if "nc" not in _CACHE:
        _CACHE["nc"] = _build()
    return _CACHE["nc"]


def _task_in_map(inputs, t, bf16, b0c, b1c, b2c):
    W0 = inputs["k0"] + SCALING * (inputs["d0"][:, :, t] @ inputs["u0"][:, :, t])
    W1 = inputs["k1"] + SCALING * (inputs["d1"][:, :, t] @ inputs["u1"][:, :, t])
    W2 = inputs["k2"] + SCALING * (inputs["d2"][:, :, t] @ inputs["u2"][:, :, t])
    # pack [K, M] -> [m, p, k*128+c] with element (m,p,kc) = W[k*128+p, m*128+c]
    w0r = np.ascontiguousarray(
        W0.reshape(8, 128, 16, 128).transpose(2, 1, 0, 3).reshape(16, 128, 1024),
        dtype=bf16,
    )
    w1r = np.ascontiguousarray(
        W1.reshape(16, 128, 16, 128).transpose(2, 1, 0, 3).reshape(16, 128, 2048),
        dtype=bf16,
    )
    w2r = np.ascontiguousarray(W2, dtype=bf16)
    xtr = np.ascontiguousarray(inputs["x"][t].T, dtype=bf16)
    return {
        "xt": xtr,
        "w0": w0r,
        "b0": b0c,
        "w1": w1r,
        "b1": b1c,
        "w2": w2r,
        "b2": b2c,
    }


def build_in_maps(inputs):
    import concurrent.futures

    import ml_dtypes

    bf16 = ml_dtypes.bfloat16
    b0c = np.ascontiguousarray(inputs["b0"].reshape(16, 128).T, dtype=np.float32)
    b1c = np.ascontiguousarray(inputs["b1"].reshape(16, 128).T, dtype=np.float32)
    b2c = np.ascontiguousarray(
        np.broadcast_to(inputs["b2"], (P, H3)), dtype=np.float32
    )
    with concurrent.futures.ThreadPoolExecutor(max_workers=T) as ex:
        in_maps = list(
            ex.map(lambda t: _task_in_map(inputs, t, bf16, b0c, b1c, b2c), range(T))
        )
    return in_maps


def kernel(**inputs):
    from concourse import bass_utils

    nc = _get_nc()
    in_maps = build_in_maps(inputs)
    res = bass_utils.run_bass_kernel_spmd(nc, in_maps, core_ids=list(range(T)))
    return np.stack([r["out"] for r in res.results], axis=0)
